# revision 25
# baseline (speedup 1.0000x reference)
"""BSC loss (single label) on 8 Trainium2 NeuronCores.

Reference computation (B=8192, H=256, C=32, T=0.1):
    f   = l2_normalize(features)                      # [B, H]
    sim = f @ f.T / T                                 # [B, B] (never materialized here)
    E   = exp(sim) with zeroed diagonal
    class_sum[i, c] = sum_{j: label_j = c} E[i, j]
    counts_excl[i, c] = counts[c] - onehot[i, c]
    denom_i = sum_c where(ce > 0, class_sum / max(ce, 1))
    mean_pos_sim_i = (sum_{j != i, same label} sim[i, j]) / P_i,  P_i = counts[l_i] - 1
    loss_i = log(max(denom_i, 1e-30)) - mean_pos_sim_i   (if P_i > 0)
    loss = sum(loss_i) / n_valid

Distribution: each core gets the inputs ROTATED by core*1024 rows and computes
the partial (sum loss_i, n_valid) over rotated rows 0..1023 (its anchor shard).
Rotation makes the program identical across cores (pure SPMD, static
addresses): anchors are always columns 0..1023 of the similarity slab and the
self-pair diagonal always falls in key blocks 0..7 at a fixed offset.

Key structure per core:
  stage A: normalize rows chunk-wise, build fT [256, 8192] (bf16) via PE
           transposes, onehot [128, 64*32] (bf16), and
           g_ext = onehot.T @ [f_norm | 1]  [32, 257] in PSUM
           (g = per-class feature sums; last column = exact class counts).
  hot loop over 64 key blocks: sim slab [128 keys, 1024 anchors] in PSUM via
           bf16 matmuls, exp(10*sim) on ACT into SBUF (bf16), zero the
           diagonal for key blocks 0..7, then accumulate class_sum [32, 1024]
           with a onehot.T @ E matmul.
  positives term needs no B^2 pass: sum_{j: label_j=c} sim[i,j] = (g @ f.T)/T
           because sim is linear in the key features.
  finale: [32, 1024] vector math + ones-vector matmul partition reductions.

The scalar partials are summed on the host (8 pairs).
"""

import numpy as np

import bass_rust
import concourse.bass as bass
import concourse.tile as tile
from concourse import mybir
from concourse.bass_utils import run_bass_kernel_spmd

F32 = mybir.dt.float32
BF16 = mybir.dt.bfloat16

B = 8192
H = 256
C = 32
N_CORES = 8
SHARD = B // N_CORES          # 1024 anchors per core
N_CHUNKS = B // 128           # 64 row chunks / key blocks
TEMP_INV = 10.0               # 1 / temperature

# Quake rsqrt seed constant (bit pattern 0x5f3759df as float32)
MAGIC_F = float(np.frombuffer(np.uint32(0x5F3759DF).tobytes(), np.float32)[0])
LN2 = float(np.log(2.0))
# cubic fit of log2(m) on [1, 2): log2(m) ~ C3 m^3 + C2 m^2 + C1 m + C0
_m = np.linspace(1.0, 2.0, 20001)
_c = np.polyfit(_m, np.log2(_m), 3)
C3, C2, C1, C0 = (float(v) for v in _c)


class SplitDrainTileContext(tile.TileContext):
    """TileContext that caps sem waits at one per instruction.

    The walrus build in this container rejects instructions carrying more
    than one sync wait ("Too many sync wait commands", e.g. on Drain and
    TensorScalarPtr). Tile freely attaches several waits per instruction, so
    split the surplus onto same-engine nops inserted immediately before the
    instruction (identical semantics: the engine blocks on every wait before
    executing it).
    """

    MAX_DRAIN_WAITS = 1

    def _lower_ordered_insts(self, ordered):
        for insts in ordered.values():
            new_list = []
            for inst in insts:
                si = inst.sync_info
                ws = list(si.on_wait) if si is not None and si.on_wait else []
                if len(ws) > 1:
                    for k, w in enumerate(ws[:-1]):
                        new_list.append(mybir.InstNoOp(
                            name=f"{inst.name}_sw{k}",
                            engine=inst.engine,
                            bass_nofuse=True,
                            sync_info=mybir.SyncInfo(on_wait=[w], on_update=[]),
                        ))
                    inst.sync_info = mybir.SyncInfo(
                        on_wait=[ws[-1]], on_update=list(si.on_update or []))
                new_list.append(inst)
            insts[:] = new_list
        super()._lower_ordered_insts(ordered)

    def _drain_and_barrier(self, tick_clock, wait_clock):
        probe = self.nc.sync.nop()
        wait_clock.add_sem_waits(
            probe.ins, bass_rust.ScopedClock({None: tick_clock.global_clock})
        )
        si = probe.ins.sync_info
        waits = list(si.on_wait) if si is not None and si.on_wait else []
        probe.ins.sync_info = bass_rust.SyncInfo(
            on_wait=waits[: self.MAX_DRAIN_WAITS], on_update=[]
        )
        for i in range(self.MAX_DRAIN_WAITS, len(waits), self.MAX_DRAIN_WAITS):
            n = self.nc.sync.nop()
            n.ins.sync_info = bass_rust.SyncInfo(
                on_wait=waits[i : i + self.MAX_DRAIN_WAITS], on_update=[]
            )
        self.nc.sync.drain()

        self.nc.all_engine_barrier()
        assert self.sems is not None
        popped = self.nc._tile_sem_poison_stack.pop()
        assert popped is self._sem_poison
        self.nc.clear_and_free_semaphores(list(self.sems.allocated().values()))
        self.nc.all_engine_barrier()


def build_program(n_iters: int = 1):
    """Emit the SPMD program. n_iters > 1 wraps the body in a hardware loop
    (identical recompute) for wall-clock timing runs."""
    nc = bass.Bass("TRN2", target_bir_lowering=False, debug=False,
                   num_devices=N_CORES)

    feat = nc.dram_tensor("feat", [B, H], F32, kind="ExternalInput")
    lab = nc.dram_tensor("lab", [128, N_CHUNKS], F32, kind="ExternalInput")
    cnt = nc.dram_tensor("cnt", [C, 1], F32, kind="ExternalInput")
    out = nc.dram_tensor("partials", [128, 2], F32, kind="ExternalOutput")

    with SplitDrainTileContext(nc) as tc:
        if n_iters == 1:
            emit_body(nc, tc, feat, lab, cnt, out)
        else:
            hints = (mybir.EngineType.PE, mybir.EngineType.Activation,
                     mybir.EngineType.DVE, mybir.EngineType.SP,
                     mybir.EngineType.Pool)
            with tc.For_i(0, n_iters, 1, hint_engines=hints):
                emit_body(nc, tc, feat, lab, cnt, out)
    return nc


def emit_body(nc, tc, feat, lab, cnt, out):
    from contextlib import ExitStack

    ACT = mybir.ActivationFunctionType
    OP = mybir.AluOpType
    AX = mybir.AxisListType

    with ExitStack() as ctx:
        ep = ctx.enter_context  # shorthand

        # ---- persistent SBUF ----
        const_pool = ep(tc.tile_pool(name="consts", bufs=1))
        identity = const_pool.tile([128, 128], BF16)
        from concourse import masks
        masks.make_identity(nc, identity[:])
        diagmask = const_pool.tile([128, 128], BF16)
        nc.gpsimd.memset(diagmask[:], 1.0)
        nc.gpsimd.affine_select(
            out=diagmask[:], in_=diagmask[:], compare_op=OP.not_equal,
            fill=0.0, base=0, pattern=[[-1, 128]], channel_multiplier=1)
        iota32 = const_pool.tile([128, C], F32)
        nc.gpsimd.iota(iota32[:], pattern=[[1, C]], base=0,
                       channel_multiplier=0,
                       allow_small_or_imprecise_dtypes=True)
        ones32 = const_pool.tile([C, 1], F32)
        nc.gpsimd.memset(ones32[:], 1.0)
        magicf = const_pool.tile([128, 8], F32)
        nc.gpsimd.memset(magicf[:], MAGIC_F)

        big_pool = ep(tc.tile_pool(name="big", bufs=1))
        fT = big_pool.tile([128, 2 * B], BF16)   # fT rows 0..127 | rows 128..255
        fT0 = fT[:, 0:B]
        fT1 = fT[:, B:2 * B]
        xb_all = big_pool.tile([128, N_CHUNKS * H], BF16)  # normalized rows
        oh_sb = big_pool.tile([128, N_CHUNKS * C], BF16)   # onehot key blocks
        lab_sb = big_pool.tile([128, N_CHUNKS], F32)
        nc.sync.dma_start(lab_sb[:], lab.ap())

        # persistent PSUM accumulator for class sums
        cs_pool = ep(tc.tile_pool(name="csacc", bufs=1, space="PSUM"))
        cs_psum = cs_pool.tile([C, SHARD], F32)  # class_sum.T for anchors

        fin = ep(tc.tile_pool(name="fin", bufs=1))

        GRP = 8   # chunks per batched-sqrt group
        LAG = 7   # hot-loop key block emitted alongside stage-A chunk kb+LAG

        # CS matmuls run two key blocks behind the sim matmuls so the PE
        # never waits on the ACT exp of the current block.
        pending = []

        def cs_mms(okb, oe):
            for nb in range(2):
                nc.tensor.matmul(
                    cs_psum[:, nb * 512:(nb + 1) * 512],
                    oh_sb[:, okb * C:(okb + 1) * C],
                    oe[:, nb * 512:(nb + 1) * 512],
                    start=(okb == 0), stop=(okb == N_CHUNKS - 1))

        with tc.tile_pool(name="simp", bufs=2, space="PSUM") as simpool, \
             tc.tile_pool(name="esb", bufs=3) as epool:

            def hot_iter(kb):
                ps = simpool.tile([128, SHARD], F32, tag="ps")
                for kc, fTk in ((0, fT0), (1, fT1)):
                    for nb in range(2):
                        nc.tensor.matmul(
                            ps[:, nb * 512:(nb + 1) * 512],
                            fTk[:, kb * 128:(kb + 1) * 128],
                            fTk[:, nb * 512:(nb + 1) * 512],
                            start=(kc == 0), stop=(kc == 1))
                if len(pending) == 2:
                    cs_mms(*pending.pop(0))
                e = epool.tile([128, SHARD], BF16, tag="e")
                nc.scalar.activation(e[:], ps[:], ACT.Exp, scale=TEMP_INV)
                if kb < SHARD // 128:
                    # self-pairs: rotated key kb*128+p vs anchor col kb*128+p
                    nc.gpsimd.tensor_tensor(
                        e[:, kb * 128:(kb + 1) * 128],
                        e[:, kb * 128:(kb + 1) * 128], diagmask[:], OP.mult)
                pending.append((kb, e))

            # ---- stage A interleaved with the first hot-loop blocks ----
            # Stage A chunk ch feeds fT columns; hot block kb needs chunks
            # <= max(kb, 7), so kb = ch - LAG is safe and keeps the PE busy
            # on similarity matmuls while DMA/DVE/ACT run the next chunks.
            with tc.tile_pool(name="gacc", bufs=1, space="PSUM") as g_pool, \
                 tc.tile_pool(name="transp", bufs=1, space="PSUM") as tp_pool, \
                 tc.tile_pool(name="xchunk", bufs=GRP + 3) as xpool, \
                 tc.tile_pool(name="sq", bufs=2) as sqpool, \
                 tc.tile_pool(name="nrm", bufs=3) as npool:
                g_psum = g_pool.tile([C, H], F32)  # onehot.T @ f_norm
                tp_big = tp_pool.tile([128, 512], BF16)
                xs = {}
                for ch in range(N_CHUNKS):
                    x = xpool.tile([128, H], F32, tag="x")
                    xs[ch] = x
                    nc.sync.dma_start(
                        x[:], feat.ap()[ch * 128:(ch + 1) * 128, :])
                    xch = x[:]
                    sq = sqpool.tile([128, H], F32, tag="sq")
                    g = ch % GRP
                    if g == 0:
                        n2g = npool.tile([128, GRP], F32, tag="n2")
                        rig = npool.tile([128, GRP], F32, tag="ri")
                    nc.vector.scalar_tensor_tensor(
                        out=sq[:], in0=xch, scalar=0.0, in1=xch,
                        op0=OP.bypass, op1=OP.mult,
                        accum_out=n2g[:, g:g + 1])
                    if g == GRP - 1:
                        # rig = rsqrt(n2g): quake seed + one Newton step.
                        # Int shift/sub on DVE, float refine on Pool; keeps
                        # the ACT engine exp-only (no table-set swaps).
                        I32 = mybir.dt.int32
                        qi = npool.tile([128, GRP], I32, tag="qi")
                        nc.vector.tensor_single_scalar(
                            qi[:], n2g[:].bitcast(I32), 1,
                            OP.arith_shift_right)
                        nc.vector.tensor_tensor(
                            rig[:].bitcast(I32), magicf[:].bitcast(I32),
                            qi[:], OP.subtract)
                        tq = npool.tile([128, GRP], F32, tag="tq")
                        nc.gpsimd.tensor_tensor(
                            tq[:], rig[:], rig[:], OP.mult)
                        nc.gpsimd.tensor_tensor(
                            tq[:], tq[:], n2g[:], OP.mult)
                        nc.gpsimd.tensor_scalar(
                            out=tq[:], in0=tq[:], scalar1=-0.5, scalar2=1.5,
                            op0=OP.mult, op1=OP.add)
                        nc.gpsimd.tensor_tensor(
                            rig[:], rig[:], tq[:], OP.mult)
                        for j in range(GRP):
                            cj = ch - (GRP - 1) + j
                            xj = xs.pop(cj)[:]
                            xbc = xb_all[:, cj * H:(cj + 1) * H]
                            nc.vector.tensor_scalar_mul(
                                xbc, xj, rig[:, j:j + 1])
                            nc.gpsimd.tensor_scalar(
                                out=oh_sb[:, cj * C:(cj + 1) * C],
                                in0=iota32[:],
                                scalar1=lab_sb[:, cj:cj + 1], scalar2=None,
                                op0=OP.is_equal)
                            nc.tensor.matmul(
                                g_psum[:], oh_sb[:, cj * C:(cj + 1) * C],
                                xbc, start=(cj == 0),
                                stop=(cj == N_CHUNKS - 1))
                            # alternate halves of one PSUM bank so chunk
                            # cj+1's transposes never wait on cj's copy
                            half = (cj % 2) * 256
                            tpc = tp_big[:, half:half + 256]
                            for kc in range(2):
                                nc.tensor.transpose(
                                    tpc[:, kc * 128:(kc + 1) * 128],
                                    xb_all[:, cj * H + kc * 128:
                                           cj * H + (kc + 1) * 128],
                                    identity[:])
                            dst = fT[:].rearrange("p (k n) -> p k n", k=2)[
                                :, :, cj * 128:(cj + 1) * 128]
                            nc.vector.tensor_copy(
                                dst, tpc.rearrange("p (k n) -> p k n", k=2))
                            if cj >= LAG:
                                hot_iter(cj - LAG)

                # ---- pre-tail finale work (independent of class sums) ----
                g_sb = fin.tile([C, H], BF16)
                nc.vector.tensor_copy(g_sb[:], g_psum[:])
                counts = fin.tile([C, 1], F32)
                nc.sync.dma_start(counts[:], cnt.ap())

                gT0 = fin.tile([128, C], BF16)
                gT1 = fin.tile([128, C], BF16)
                ohT = fin.tile([C, SHARD], F32)
                for kc, gTk in ((0, gT0), (1, gT1)):
                    tpg = tp_big[:, (kc % 2) * 256:(kc % 2) * 256 + 256]
                    nc.tensor.transpose(
                        tpg[:, 0:C], g_sb[:, kc * 128:(kc + 1) * 128],
                        identity[0:C, 0:C])
                    nc.vector.tensor_copy(gTk[:], tpg[:, 0:C])
                for bkl in range(SHARD // 128):
                    tpo = tp_big[:, (bkl % 2) * 256:(bkl % 2) * 256 + 256]
                    nc.tensor.transpose(
                        tpo[0:C, 0:128], oh_sb[:, bkl * C:(bkl + 1) * C],
                        identity[:])
                    nc.vector.tensor_copy(
                        ohT[:, bkl * 128:(bkl + 1) * 128], tpo[0:C, 0:128])

            with tc.tile_pool(name="rpsum", bufs=1, space="PSUM") as rpool:
                r_psum = rpool.tile([C, SHARD], F32)
                for kc, (gTk, fTk) in enumerate(((gT0, fT0), (gT1, fT1))):
                    for nb in range(2):
                        nc.tensor.matmul(
                            r_psum[:, nb * 512:(nb + 1) * 512], gTk[:],
                            fTk[:, nb * 512:(nb + 1) * 512],
                            start=(kc == 0), stop=(kc == 1))

                # counts_excl, masks, positives numerator / denominator
                ce = fin.tile([C, SHARD], F32)
                nc.vector.tensor_scalar(
                    out=ce[:], in0=ohT[:], scalar1=counts[:], scalar2=-1.0,
                    op0=OP.subtract, op1=OP.mult)
                mask = fin.tile([C, SHARD], F32)
                nc.vector.tensor_single_scalar(mask[:], ce[:], 0.5, OP.is_gt)
                ce1 = fin.tile([C, SHARD], F32)
                nc.vector.tensor_single_scalar(ce1[:], ce[:], 1.0, OP.max)
                rce = fin.tile([C, SHARD], F32)
                nc.vector.reciprocal(rce[:], ce1[:])
                nc.vector.tensor_tensor(rce[:], rce[:], mask[:], OP.mult)

                pnum = fin.tile([C, SHARD], F32)
                nc.vector.scalar_tensor_tensor(
                    out=pnum[:], in0=r_psum[:], scalar=1.0, in1=ohT[:],
                    op0=OP.subtract, op1=OP.mult)
                pden = fin.tile([C, SHARD], F32)
                nc.vector.tensor_tensor(pden[:], ohT[:], ce[:], OP.mult)

            # Row reductions land in [1, 1024] PSUM; a small DMA reshapes
            # them to [128, 8] SBUF so the pointwise tail runs 128-wide.
            with tc.tile_pool(name="rows_pre", bufs=1, space="PSUM") as rowp:
                pv = fin.tile([128, 8], F32)    # P_i (positive count)
                pos = fin.tile([128, 8], F32)   # sum_pos raw sim
                for src, dst in ((pden, pv), (pnum, pos)):
                    row = rowp.tile([1, SHARD], F32, tag="row")
                    for nb in range(2):
                        nc.tensor.matmul(
                            row[:, nb * 512:(nb + 1) * 512], ones32[:],
                            src[:, nb * 512:(nb + 1) * 512],
                            start=True, stop=True)
                    rsb = fin.tile([1, SHARD], F32)
                    nc.vector.tensor_copy(rsb[:], row[:])
                    nc.sync.dma_start(dst[:], rsb[:])

                valid = fin.tile([128, 8], F32)
                nc.vector.tensor_single_scalar(
                    valid[:], pv[:], 0.5, OP.is_gt)
                nc.vector.tensor_single_scalar(pv[:], pv[:], 1.0, OP.max)
                rp = fin.tile([128, 8], F32)
                nc.vector.reciprocal(rp[:], pv[:])
                mp = fin.tile([128, 8], F32)
                nc.vector.scalar_tensor_tensor(
                    out=mp[:], in0=pos[:], scalar=TEMP_INV, in1=rp[:],
                    op0=OP.mult, op1=OP.mult)

            # ---- remaining hot-loop blocks ----
            for kb in range(N_CHUNKS - LAG, N_CHUNKS):
                hot_iter(kb)
            while pending:
                cs_mms(*pending.pop(0))

        # ---- post-hot finale: denominator path and output ----
        with tc.tile_pool(name="rows_post", bufs=1, space="PSUM") as rowpool:
            terms = fin.tile([C, SHARD], F32)
            nc.vector.tensor_tensor(terms[:], cs_psum[:], rce[:], OP.mult)
            row = rowpool.tile([1, SHARD], F32)
            for nb in range(2):
                nc.tensor.matmul(
                    row[:, nb * 512:(nb + 1) * 512], ones32[:],
                    terms[:, nb * 512:(nb + 1) * 512],
                    start=True, stop=True)
            rsb2 = fin.tile([1, SHARD], F32)
            nc.vector.tensor_copy(rsb2[:], row[:])
            dv = fin.tile([128, 8], F32)
            nc.sync.dma_start(dv[:], rsb2[:])

            # ln(dv) without touching the ACT table: exponent/mantissa
            # split + cubic log2(m) on DVE. denom >= exp(-10)/1024 whenever
            # valid, so no zero guard is needed (invalid rows are masked).
            I32 = mybir.dt.int32
            ei = fin.tile([128, 8], I32)
            nc.vector.tensor_single_scalar(
                ei[:], dv[:].bitcast(I32), 23, OP.logical_shift_right)
            ef = fin.tile([128, 8], F32)
            nc.vector.tensor_copy(ef[:], ei[:])
            mi = fin.tile([128, 8], I32)
            nc.vector.tensor_single_scalar(
                mi[:], dv[:].bitcast(I32), 0x007FFFFF, OP.bitwise_and)
            nc.vector.tensor_single_scalar(
                mi[:], mi[:], 0x3F800000, OP.bitwise_or)
            mf = mi[:].bitcast(F32)
            lp = fin.tile([128, 8], F32)
            nc.vector.scalar_tensor_tensor(
                out=lp[:], in0=mf, scalar=C2 / C3, in1=mf,
                op0=OP.add, op1=OP.mult)
            nc.vector.scalar_tensor_tensor(
                out=lp[:], in0=lp[:], scalar=C1 / C3, in1=mf,
                op0=OP.add, op1=OP.mult)
            nc.vector.tensor_scalar(
                out=lp[:], in0=lp[:], scalar1=C3, scalar2=C0 - 127.0,
                op0=OP.mult, op1=OP.add)
            nc.vector.tensor_tensor(lp[:], lp[:], ef[:], OP.add)
            # li = (ln2 * log2(denom) - mp) * valid
            nc.vector.scalar_tensor_tensor(
                out=lp[:], in0=lp[:], scalar=LN2, in1=mp[:],
                op0=OP.mult, op1=OP.subtract)
            nc.vector.tensor_tensor(lp[:], lp[:], valid[:], OP.mult)

            res = fin.tile([128, 2], F32)
            nc.vector.tensor_reduce(res[:, 0:1], lp[:], axis=AX.X, op=OP.add)
            nc.vector.tensor_reduce(res[:, 1:2], valid[:], axis=AX.X,
                                    op=OP.add)
            nc.sync.dma_start(out.ap(), res[:])


_PROGRAM_CACHE = {}


def get_program(n_iters: int = 1):
    if n_iters not in _PROGRAM_CACHE:
        _PROGRAM_CACHE[n_iters] = build_program(n_iters)
    return _PROGRAM_CACHE[n_iters]


def make_in_maps(features: np.ndarray, labels: np.ndarray):
    features = np.ascontiguousarray(np.asarray(features, dtype=np.float32))
    labels_i = np.asarray(labels).astype(np.int64)
    labels_f = labels_i.astype(np.float32)
    counts = np.bincount(labels_i, minlength=C).astype(np.float32)
    cnt = np.ascontiguousarray(counts.reshape(C, 1))
    in_maps = []
    for c in range(N_CORES):
        fr = np.roll(features, -c * SHARD, axis=0)
        lr = np.roll(labels_f, -c * SHARD)
        in_maps.append({
            "feat": np.ascontiguousarray(fr),
            "lab": np.ascontiguousarray(lr.reshape(N_CHUNKS, 128).T),
            "cnt": cnt,
        })
    return in_maps


def kernel(features, labels):
    nc = get_program(1)
    in_maps = make_in_maps(features, labels)
    res = run_bass_kernel_spmd(nc, in_maps, list(range(N_CORES)))
    loss_sum = np.float64(0.0)
    n_valid = np.float64(0.0)
    for c in range(N_CORES):
        p = res.results[c]["partials"]
        loss_sum += np.float64(p[:, 0].sum(dtype=np.float64))
        n_valid += np.float64(p[:, 1].sum(dtype=np.float64))
    if n_valid > 0:
        loss = loss_sum / np.float32(max(n_valid, 1.0))
    else:
        loss = np.float32(0.0)
    return np.array(loss, dtype=np.float32)



# revision 27
# speedup vs baseline: 1.3481x; 1.3481x over previous
"""BSC loss (single label) on 8 Trainium2 NeuronCores.

Reference computation (B=8192, H=256, C=32, T=0.1):
    f   = l2_normalize(features)                      # [B, H]
    sim = f @ f.T / T                                 # [B, B] (never materialized here)
    E   = exp(sim) with zeroed diagonal
    class_sum[i, c] = sum_{j: label_j = c} E[i, j]
    counts_excl[i, c] = counts[c] - onehot[i, c]
    denom_i = sum_c where(ce > 0, class_sum / max(ce, 1))
    mean_pos_sim_i = (sum_{j != i, same label} sim[i, j]) / P_i,  P_i = counts[l_i] - 1
    loss_i = log(max(denom_i, 1e-30)) - mean_pos_sim_i   (if P_i > 0)
    loss = sum(loss_i) / n_valid

Distribution: each core gets the inputs ROTATED by core*1024 rows and computes
the partial (sum loss_i, n_valid) over rotated rows 0..1023 (its anchor shard).
Rotation makes the program identical across cores (pure SPMD, static
addresses): anchors are always columns 0..1023 of the similarity slab and the
self-pair diagonal always falls in key blocks 0..7 at a fixed offset.

Key structure per core:
  stage A: normalize rows chunk-wise, build fT [256, 8192] (bf16) via PE
           transposes, onehot [128, 64*32] (bf16), and
           g_ext = onehot.T @ [f_norm | 1]  [32, 257] in PSUM
           (g = per-class feature sums; last column = exact class counts).
  hot loop over 64 key blocks: sim slab [128 keys, 1024 anchors] in PSUM via
           bf16 matmuls, exp(10*sim) on ACT into SBUF (bf16), zero the
           diagonal for key blocks 0..7, then accumulate class_sum [32, 1024]
           with a onehot.T @ E matmul.
  positives term needs no B^2 pass: sum_{j: label_j=c} sim[i,j] = (g @ f.T)/T
           because sim is linear in the key features.
  finale: [32, 1024] vector math + ones-vector matmul partition reductions.

The scalar partials are summed on the host (8 pairs).
"""

import numpy as np

import bass_rust
import concourse.bass as bass
import concourse.tile as tile
from concourse import mybir
from concourse.bass_utils import run_bass_kernel_spmd

F32 = mybir.dt.float32
BF16 = mybir.dt.bfloat16
FP8 = mybir.dt.float8e4
FP8_SCALE = 16.0              # fT stored as 16*f_norm in fp8e4

B = 8192
H = 256
C = 32
N_CORES = 8
SHARD = B // N_CORES          # 1024 anchors per core
N_CHUNKS = B // 128           # 64 row chunks / key blocks
TEMP_INV = 10.0               # 1 / temperature

# Quake rsqrt seed constant (bit pattern 0x5f3759df as float32)
MAGIC_F = float(np.frombuffer(np.uint32(0x5F3759DF).tobytes(), np.float32)[0])
LN2 = float(np.log(2.0))
# cubic fit of log2(m) on [1, 2): log2(m) ~ C3 m^3 + C2 m^2 + C1 m + C0
_m = np.linspace(1.0, 2.0, 20001)
_c = np.polyfit(_m, np.log2(_m), 3)
C3, C2, C1, C0 = (float(v) for v in _c)


class SplitDrainTileContext(tile.TileContext):
    """TileContext that caps sem waits at one per instruction.

    The walrus build in this container rejects instructions carrying more
    than one sync wait ("Too many sync wait commands", e.g. on Drain and
    TensorScalarPtr). Tile freely attaches several waits per instruction, so
    split the surplus onto same-engine nops inserted immediately before the
    instruction (identical semantics: the engine blocks on every wait before
    executing it).
    """

    MAX_DRAIN_WAITS = 1

    def _lower_ordered_insts(self, ordered):
        for insts in ordered.values():
            new_list = []
            for inst in insts:
                si = inst.sync_info
                ws = list(si.on_wait) if si is not None and si.on_wait else []
                if len(ws) > 1:
                    for k, w in enumerate(ws[:-1]):
                        new_list.append(mybir.InstNoOp(
                            name=f"{inst.name}_sw{k}",
                            engine=inst.engine,
                            bass_nofuse=True,
                            sync_info=mybir.SyncInfo(on_wait=[w], on_update=[]),
                        ))
                    inst.sync_info = mybir.SyncInfo(
                        on_wait=[ws[-1]], on_update=list(si.on_update or []))
                new_list.append(inst)
            insts[:] = new_list
        super()._lower_ordered_insts(ordered)

    def _drain_and_barrier(self, tick_clock, wait_clock):
        probe = self.nc.sync.nop()
        wait_clock.add_sem_waits(
            probe.ins, bass_rust.ScopedClock({None: tick_clock.global_clock})
        )
        si = probe.ins.sync_info
        waits = list(si.on_wait) if si is not None and si.on_wait else []
        probe.ins.sync_info = bass_rust.SyncInfo(
            on_wait=waits[: self.MAX_DRAIN_WAITS], on_update=[]
        )
        for i in range(self.MAX_DRAIN_WAITS, len(waits), self.MAX_DRAIN_WAITS):
            n = self.nc.sync.nop()
            n.ins.sync_info = bass_rust.SyncInfo(
                on_wait=waits[i : i + self.MAX_DRAIN_WAITS], on_update=[]
            )
        self.nc.sync.drain()

        self.nc.all_engine_barrier()
        assert self.sems is not None
        popped = self.nc._tile_sem_poison_stack.pop()
        assert popped is self._sem_poison
        self.nc.clear_and_free_semaphores(list(self.sems.allocated().values()))
        self.nc.all_engine_barrier()


def build_program(n_iters: int = 1):
    """Emit the SPMD program. n_iters > 1 wraps the body in a hardware loop
    (identical recompute) for wall-clock timing runs."""
    nc = bass.Bass("TRN2", target_bir_lowering=False, debug=False,
                   num_devices=N_CORES)

    feat = nc.dram_tensor("feat", [B, H], F32, kind="ExternalInput")
    lab = nc.dram_tensor("lab", [128, N_CHUNKS], F32, kind="ExternalInput")
    cnt = nc.dram_tensor("cnt", [C, 1], F32, kind="ExternalInput")
    out = nc.dram_tensor("partials", [128, 2], F32, kind="ExternalOutput")

    with SplitDrainTileContext(nc) as tc:
        if n_iters == 1:
            emit_body(nc, tc, feat, lab, cnt, out)
        else:
            hints = (mybir.EngineType.PE, mybir.EngineType.Activation,
                     mybir.EngineType.DVE, mybir.EngineType.SP,
                     mybir.EngineType.Pool)
            with tc.For_i(0, n_iters, 1, hint_engines=hints):
                emit_body(nc, tc, feat, lab, cnt, out)
    return nc


def emit_body(nc, tc, feat, lab, cnt, out):
    from contextlib import ExitStack

    ACT = mybir.ActivationFunctionType
    OP = mybir.AluOpType
    AX = mybir.AxisListType

    with ExitStack() as ctx:
        ep = ctx.enter_context  # shorthand

        # ---- persistent SBUF ----
        const_pool = ep(tc.tile_pool(name="consts", bufs=1))
        identity = const_pool.tile([128, 128], BF16)
        from concourse import masks
        masks.make_identity(nc, identity[:])
        diagmask = const_pool.tile([128, 128], BF16)
        nc.gpsimd.memset(diagmask[:], 1.0)
        nc.gpsimd.affine_select(
            out=diagmask[:], in_=diagmask[:], compare_op=OP.not_equal,
            fill=0.0, base=0, pattern=[[-1, 128]], channel_multiplier=1)
        iota32 = const_pool.tile([128, C], F32)
        nc.gpsimd.iota(iota32[:], pattern=[[1, C]], base=0,
                       channel_multiplier=0,
                       allow_small_or_imprecise_dtypes=True)
        ones32 = const_pool.tile([C, 1], F32)
        nc.gpsimd.memset(ones32[:], 1.0)
        magicf = const_pool.tile([128, 8], F32)
        nc.gpsimd.memset(magicf[:], MAGIC_F)

        big_pool = ep(tc.tile_pool(name="big", bufs=1))
        # fT stored fp8e4 as 16*f_norm; plane k holds rows k*128..k*128+127
        fT = big_pool.tile([128, 2 * B], FP8)
        fTv = fT[:].rearrange("p (k n) -> p k n", k=2)
        oh_sb = big_pool.tile([128, N_CHUNKS * C], BF16)   # onehot key blocks
        lab_sb = big_pool.tile([128, N_CHUNKS], F32)
        nc.sync.dma_start(lab_sb[:], lab.ap())

        # persistent PSUM accumulator for class sums
        cs_pool = ep(tc.tile_pool(name="csacc", bufs=1, space="PSUM"))
        cs_psum = cs_pool.tile([C, SHARD], F32)  # class_sum.T for anchors

        fin = ep(tc.tile_pool(name="fin", bufs=1))

        GRP = 8   # chunks per batched-sqrt group
        LAG = 7   # hot-loop key block emitted alongside stage-A chunk kb+LAG

        # CS matmuls run two key blocks behind the sim matmuls so the PE
        # never waits on the ACT exp of the current block.
        pending = []

        def cs_mms(okb, oe):
            for nb in range(2):
                nc.tensor.matmul(
                    cs_psum[:, nb * 512:(nb + 1) * 512],
                    oh_sb[:, okb * C:(okb + 1) * C],
                    oe[:, nb * 512:(nb + 1) * 512],
                    start=(okb == 0), stop=(okb == N_CHUNKS - 1))

        with tc.tile_pool(name="simp", bufs=2, space="PSUM") as simpool, \
             tc.tile_pool(name="esb", bufs=3) as epool:

            def hot_iter(kb):
                ps = simpool.tile([128, SHARD], F32, tag="ps")
                for nb in range(2):
                    nc.tensor.matmul(
                        ps[:, nb * 512:(nb + 1) * 512],
                        fTv[:, :, kb * 128:(kb + 1) * 128],
                        fTv[:, :, nb * 512:(nb + 1) * 512],
                        start=True, stop=True,
                        perf_mode=mybir.MatmulPerfMode.DoubleRow)
                if len(pending) == 2:
                    cs_mms(*pending.pop(0))
                e = epool.tile([128, SHARD], BF16, tag="e")
                nc.scalar.activation(e[:], ps[:], ACT.Exp,
                     scale=TEMP_INV / (FP8_SCALE * FP8_SCALE))
                if kb < SHARD // 128:
                    # self-pairs: rotated key kb*128+p vs anchor col kb*128+p
                    nc.gpsimd.tensor_tensor(
                        e[:, kb * 128:(kb + 1) * 128],
                        e[:, kb * 128:(kb + 1) * 128], diagmask[:], OP.mult)
                pending.append((kb, e))

            # ---- stage A interleaved with the first hot-loop blocks ----
            # Stage A chunk ch feeds fT columns; hot block kb needs chunks
            # <= max(kb, 7), so kb = ch - LAG is safe and keeps the PE busy
            # on similarity matmuls while DMA/DVE/ACT run the next chunks.
            with tc.tile_pool(name="gacc", bufs=1, space="PSUM") as g_pool, \
                 tc.tile_pool(name="transp", bufs=1, space="PSUM") as tp_pool, \
                 tc.tile_pool(name="xchunk", bufs=GRP + 3) as xpool, \
                 tc.tile_pool(name="sq", bufs=2) as sqpool, \
                 tc.tile_pool(name="nrm", bufs=3) as npool:
                g_psum = g_pool.tile([C, H], F32)  # onehot.T @ f_norm
                tp_big = tp_pool.tile([128, 512], BF16)
                xs = {}
                for ch in range(N_CHUNKS):
                    x = xpool.tile([128, H], F32, tag="x")
                    xs[ch] = x
                    nc.sync.dma_start(
                        x[:], feat.ap()[ch * 128:(ch + 1) * 128, :])
                    xch = x[:]
                    sq = sqpool.tile([128, H], F32, tag="sq")
                    g = ch % GRP
                    if g == 0:
                        n2g = npool.tile([128, GRP], F32, tag="n2")
                        rig = npool.tile([128, GRP], F32, tag="ri")
                    nc.vector.scalar_tensor_tensor(
                        out=sq[:], in0=xch, scalar=0.0, in1=xch,
                        op0=OP.bypass, op1=OP.mult,
                        accum_out=n2g[:, g:g + 1])
                    if g == GRP - 1:
                        # rig = rsqrt(n2g): quake seed + one Newton step.
                        # Int shift/sub on DVE, float refine on Pool; keeps
                        # the ACT engine exp-only (no table-set swaps).
                        I32 = mybir.dt.int32
                        qi = npool.tile([128, GRP], I32, tag="qi")
                        nc.vector.tensor_single_scalar(
                            qi[:], n2g[:].bitcast(I32), 1,
                            OP.arith_shift_right)
                        nc.vector.tensor_tensor(
                            rig[:].bitcast(I32), magicf[:].bitcast(I32),
                            qi[:], OP.subtract)
                        tq = npool.tile([128, GRP], F32, tag="tq")
                        nc.gpsimd.tensor_tensor(
                            tq[:], rig[:], rig[:], OP.mult)
                        nc.gpsimd.tensor_tensor(
                            tq[:], tq[:], n2g[:], OP.mult)
                        nc.gpsimd.tensor_scalar(
                            out=tq[:], in0=tq[:], scalar1=-0.5, scalar2=1.5,
                            op0=OP.mult, op1=OP.add)
                        nc.gpsimd.tensor_tensor(
                            rig[:], rig[:], tq[:], OP.mult)
                        for j in range(GRP):
                            cj = ch - (GRP - 1) + j
                            xj = xs.pop(cj)[:]
                            xbt = xpool.tile([128, H], BF16, tag="xb")
                            xbc = xbt[:]
                            nc.vector.tensor_scalar_mul(
                                xbc, xj, rig[:, j:j + 1])
                            nc.gpsimd.tensor_scalar(
                                out=oh_sb[:, cj * C:(cj + 1) * C],
                                in0=iota32[:],
                                scalar1=lab_sb[:, cj:cj + 1], scalar2=None,
                                op0=OP.is_equal)
                            nc.tensor.matmul(
                                g_psum[:], oh_sb[:, cj * C:(cj + 1) * C],
                                xbc, start=(cj == 0),
                                stop=(cj == N_CHUNKS - 1))
                            # alternate halves of one PSUM bank so chunk
                            # cj+1's transposes never wait on cj's copy
                            half = (cj % 2) * 256
                            tpc = tp_big[:, half:half + 256]
                            for kc in range(2):
                                nc.tensor.transpose(
                                    tpc[:, kc * 128:(kc + 1) * 128],
                                    xbc[:, kc * 128:(kc + 1) * 128],
                                    identity[:])
                            dst = fTv[:, :, cj * 128:(cj + 1) * 128]
                            nc.vector.tensor_single_scalar(
                                dst, tpc.rearrange("p (k n) -> p k n", k=2),
                                FP8_SCALE, OP.mult)
                            if cj >= LAG:
                                hot_iter(cj - LAG)

                # ---- pre-tail finale work (independent of class sums) ----
                g_sb = fin.tile([C, H], BF16)
                nc.vector.tensor_copy(g_sb[:], g_psum[:])
                counts = fin.tile([C, 1], F32)
                nc.sync.dma_start(counts[:], cnt.ap())

                gT8 = fin.tile([128, 2 * C], FP8)
                gT8v = gT8[:].rearrange("p (k n) -> p k n", k=2)
                ohT = fin.tile([C, SHARD], F32)
                for kc in range(2):
                    tpg = tp_big[:, (kc % 2) * 256:(kc % 2) * 256 + 256]
                    nc.tensor.transpose(
                        tpg[:, 0:C], g_sb[:, kc * 128:(kc + 1) * 128],
                        identity[0:C, 0:C])
                    nc.vector.tensor_single_scalar(
                        gT8[:, kc * C:(kc + 1) * C], tpg[:, 0:C],
                        FP8_SCALE, OP.mult)
                for bkl in range(SHARD // 128):
                    tpo = tp_big[:, (bkl % 2) * 256:(bkl % 2) * 256 + 256]
                    nc.tensor.transpose(
                        tpo[0:C, 0:128], oh_sb[:, bkl * C:(bkl + 1) * C],
                        identity[:])
                    nc.vector.tensor_copy(
                        ohT[:, bkl * 128:(bkl + 1) * 128], tpo[0:C, 0:128])

            with tc.tile_pool(name="rpsum", bufs=1, space="PSUM") as rpool:
                r_psum = rpool.tile([C, SHARD], F32)
                for nb in range(2):
                    nc.tensor.matmul(
                        r_psum[:, nb * 512:(nb + 1) * 512], gT8v[:],
                        fTv[:, :, nb * 512:(nb + 1) * 512],
                        start=True, stop=True,
                        perf_mode=mybir.MatmulPerfMode.DoubleRow)

                # counts_excl, masks, positives numerator / denominator
                ce = fin.tile([C, SHARD], F32)
                nc.vector.tensor_scalar(
                    out=ce[:], in0=ohT[:], scalar1=counts[:], scalar2=-1.0,
                    op0=OP.subtract, op1=OP.mult)
                mask = fin.tile([C, SHARD], F32)
                nc.vector.tensor_single_scalar(mask[:], ce[:], 0.5, OP.is_gt)
                ce1 = fin.tile([C, SHARD], F32)
                nc.vector.tensor_single_scalar(ce1[:], ce[:], 1.0, OP.max)
                rce = fin.tile([C, SHARD], F32)
                nc.vector.reciprocal(rce[:], ce1[:])
                nc.vector.tensor_tensor(rce[:], rce[:], mask[:], OP.mult)

                pnum = fin.tile([C, SHARD], F32)
                nc.vector.scalar_tensor_tensor(
                    out=pnum[:], in0=r_psum[:],
                    scalar=FP8_SCALE * FP8_SCALE, in1=ohT[:],
                    op0=OP.subtract, op1=OP.mult)
                pden = fin.tile([C, SHARD], F32)
                nc.vector.tensor_tensor(pden[:], ohT[:], ce[:], OP.mult)

            # Row reductions land in [1, 1024] PSUM; a small DMA reshapes
            # them to [128, 8] SBUF so the pointwise tail runs 128-wide.
            with tc.tile_pool(name="rows_pre", bufs=1, space="PSUM") as rowp:
                pv = fin.tile([128, 8], F32)    # P_i (positive count)
                pos = fin.tile([128, 8], F32)   # sum_pos raw sim
                for src, dst in ((pden, pv), (pnum, pos)):
                    row = rowp.tile([1, SHARD], F32, tag="row")
                    for nb in range(2):
                        nc.tensor.matmul(
                            row[:, nb * 512:(nb + 1) * 512], ones32[:],
                            src[:, nb * 512:(nb + 1) * 512],
                            start=True, stop=True)
                    rsb = fin.tile([1, SHARD], F32)
                    nc.vector.tensor_copy(rsb[:], row[:])
                    nc.sync.dma_start(dst[:], rsb[:])

                valid = fin.tile([128, 8], F32)
                nc.vector.tensor_single_scalar(
                    valid[:], pv[:], 0.5, OP.is_gt)
                nc.vector.tensor_single_scalar(pv[:], pv[:], 1.0, OP.max)
                rp = fin.tile([128, 8], F32)
                nc.vector.reciprocal(rp[:], pv[:])
                mp = fin.tile([128, 8], F32)
                nc.vector.scalar_tensor_tensor(
                    out=mp[:], in0=pos[:],
                    scalar=TEMP_INV / (FP8_SCALE * FP8_SCALE), in1=rp[:],
                    op0=OP.mult, op1=OP.mult)

            # ---- remaining hot-loop blocks ----
            for kb in range(N_CHUNKS - LAG, N_CHUNKS):
                hot_iter(kb)
            while pending:
                cs_mms(*pending.pop(0))

        # ---- post-hot finale: denominator path and output ----
        with tc.tile_pool(name="rows_post", bufs=1, space="PSUM") as rowpool:
            terms = fin.tile([C, SHARD], F32)
            nc.vector.tensor_tensor(terms[:], cs_psum[:], rce[:], OP.mult)
            row = rowpool.tile([1, SHARD], F32)
            for nb in range(2):
                nc.tensor.matmul(
                    row[:, nb * 512:(nb + 1) * 512], ones32[:],
                    terms[:, nb * 512:(nb + 1) * 512],
                    start=True, stop=True)
            rsb2 = fin.tile([1, SHARD], F32)
            nc.vector.tensor_copy(rsb2[:], row[:])
            dv = fin.tile([128, 8], F32)
            nc.sync.dma_start(dv[:], rsb2[:])

            # ln(dv) without touching the ACT table: exponent/mantissa
            # split + cubic log2(m) on DVE. denom >= exp(-10)/1024 whenever
            # valid, so no zero guard is needed (invalid rows are masked).
            I32 = mybir.dt.int32
            ei = fin.tile([128, 8], I32)
            nc.vector.tensor_single_scalar(
                ei[:], dv[:].bitcast(I32), 23, OP.logical_shift_right)
            ef = fin.tile([128, 8], F32)
            nc.vector.tensor_copy(ef[:], ei[:])
            mi = fin.tile([128, 8], I32)
            nc.vector.tensor_single_scalar(
                mi[:], dv[:].bitcast(I32), 0x007FFFFF, OP.bitwise_and)
            nc.vector.tensor_single_scalar(
                mi[:], mi[:], 0x3F800000, OP.bitwise_or)
            mf = mi[:].bitcast(F32)
            lp = fin.tile([128, 8], F32)
            nc.vector.scalar_tensor_tensor(
                out=lp[:], in0=mf, scalar=C2 / C3, in1=mf,
                op0=OP.add, op1=OP.mult)
            nc.vector.scalar_tensor_tensor(
                out=lp[:], in0=lp[:], scalar=C1 / C3, in1=mf,
                op0=OP.add, op1=OP.mult)
            nc.vector.tensor_scalar(
                out=lp[:], in0=lp[:], scalar1=C3, scalar2=C0 - 127.0,
                op0=OP.mult, op1=OP.add)
            nc.vector.tensor_tensor(lp[:], lp[:], ef[:], OP.add)
            # li = (ln2 * log2(denom) - mp) * valid
            nc.vector.scalar_tensor_tensor(
                out=lp[:], in0=lp[:], scalar=LN2, in1=mp[:],
                op0=OP.mult, op1=OP.subtract)
            nc.vector.tensor_tensor(lp[:], lp[:], valid[:], OP.mult)

            res = fin.tile([128, 2], F32)
            nc.vector.tensor_reduce(res[:, 0:1], lp[:], axis=AX.X, op=OP.add)
            nc.vector.tensor_reduce(res[:, 1:2], valid[:], axis=AX.X,
                                    op=OP.add)
            nc.sync.dma_start(out.ap(), res[:])


_PROGRAM_CACHE = {}


def get_program(n_iters: int = 1):
    if n_iters not in _PROGRAM_CACHE:
        _PROGRAM_CACHE[n_iters] = build_program(n_iters)
    return _PROGRAM_CACHE[n_iters]


def make_in_maps(features: np.ndarray, labels: np.ndarray):
    features = np.ascontiguousarray(np.asarray(features, dtype=np.float32))
    labels_i = np.asarray(labels).astype(np.int64)
    labels_f = labels_i.astype(np.float32)
    counts = np.bincount(labels_i, minlength=C).astype(np.float32)
    cnt = np.ascontiguousarray(counts.reshape(C, 1))
    in_maps = []
    for c in range(N_CORES):
        fr = np.roll(features, -c * SHARD, axis=0)
        lr = np.roll(labels_f, -c * SHARD)
        in_maps.append({
            "feat": np.ascontiguousarray(fr),
            "lab": np.ascontiguousarray(lr.reshape(N_CHUNKS, 128).T),
            "cnt": cnt,
        })
    return in_maps


def kernel(features, labels):
    nc = get_program(1)
    in_maps = make_in_maps(features, labels)
    res = run_bass_kernel_spmd(nc, in_maps, list(range(N_CORES)))
    loss_sum = np.float64(0.0)
    n_valid = np.float64(0.0)
    for c in range(N_CORES):
        p = res.results[c]["partials"]
        loss_sum += np.float64(p[:, 0].sum(dtype=np.float64))
        n_valid += np.float64(p[:, 1].sum(dtype=np.float64))
    if n_valid > 0:
        loss = loss_sum / np.float32(max(n_valid, 1.0))
    else:
        loss = np.float32(0.0)
    return np.array(loss, dtype=np.float32)

